# revision 7
# baseline (speedup 1.0000x reference)
"""CeATTForTCPFormer Trainium2 kernel (8 NeuronCores, data-parallel over B).

Contract: kernel(**inputs) takes FULL inputs as in reference.setup_inputs()
and returns the FULL [32, 243, 17, 256] fp32 output. Internally shards B
across 8 cores (4 per core); BN batch stats are combined with one small
AllReduce.

v2: all DMAs are contiguous/large. x is pre-transposed to channel-major
bf16 on host; output is stored channel-major f32 and re-transposed on
host. Spatial QKV is produced seq-major directly by per-l matmuls (no
DRAM fold), attention output returns to channel-major via PE transposes,
pre-BN activations are parked in SBUF, and f_w2/p_w are fused host-side.
"""
import os
import sys

sys.path.insert(0, "/opt/trn_rl_repo")
sys.path.insert(0, "/opt/trn_rl_repo/concourse")

import numpy as np
import ml_dtypes

import concourse.bass as bass
import concourse.mybir as mybir
import concourse.tile as tile
from concourse.bass_utils import run_bass_kernel_spmd

F32 = mybir.dt.float32
BF16 = mybir.dt.bfloat16
AF = mybir.ActivationFunctionType
OP = mybir.AluOpType
AX = mybir.AxisListType

N_CORES = 8
B, T, J, C = 32, 243, 17, 256
BSH = B // N_CORES          # 4 batch elems per core
H, DH = 8, 32
LT = 81                     # temporal pooled length (243/3)
LS = 8                      # spatial pooled length (17//2)
NT_B = J                    # temporal seqs per batch elem
TOKT_B = J * LT             # 1377 temporal tokens per b
NS_B = T                    # spatial seqs per b (243)
TOKS_B = T * LS             # 1944 spatial tokens per b
NTOK = T * J                # 4131 fusion tokens per b
CNT_T = float(B * J * LT)   # global BN count temporal = 44064
CNT_S = float(B * T * LS)   # spatial N = B*T, L = 8 -> 62208
SCALE = 1.0 / np.sqrt(DH)
EPS = 1e-5

MAX_WAITS = 1


def _split_excess_waits(nc):
    ctr = 0
    for f in nc.m.functions:
        for bb in f.blocks:
            new_insts, changed = [], False
            for inst in bb.instructions:
                si = inst.sync_info
                if si is not None and si.on_wait is not None and len(si.on_wait) > MAX_WAITS:
                    waits = list(si.on_wait)
                    upd = list(si.on_update or [])
                    rest, keep = waits[:-MAX_WAITS], waits[-MAX_WAITS:]
                    for w in rest:
                        nop = mybir.InstNoOp(name=f"waitsplit_{ctr}", ins=[], outs=[])
                        ctr += 1
                        nop.engine = inst.engine
                        nop.sync_info = mybir.SyncInfo(on_wait=[w], on_update=[])
                        new_insts.append(nop)
                    inst.sync_info = mybir.SyncInfo(on_wait=keep, on_update=upd)
                    changed = True
                new_insts.append(inst)
            if changed:
                bb.instructions = new_insts


def _interp_lin_coef(L, out_len):
    scale = L / out_len
    coords = (np.arange(out_len) + 0.5) * scale - 0.5
    coords = np.clip(coords, 0.0, L - 1)
    lo = np.floor(coords).astype(np.int32)
    hi = np.minimum(lo + 1, L - 1)
    w = (coords - lo).astype(np.float64)
    return lo, hi, w


PHASES = int(os.environ.get("KPHASES", "4"))


def build(nc, dbg=()):
    dbg = set(dbg)
    dbg_outs = []

    x_t = nc.dram_tensor("xs", [C, BSH * NTOK], BF16, kind="ExternalInput").ap()
    wqkv_t = nc.dram_tensor("wqkv_t", [C, 3 * C], BF16, kind="ExternalInput").ap()
    wqkv_s = nc.dram_tensor("wqkv_s", [C, 3 * C], BF16, kind="ExternalInput").ap()
    wproj_t = nc.dram_tensor("wproj_t", [C, C], BF16, kind="ExternalInput").ap()
    wproj_s = nc.dram_tensor("wproj_s", [C, C], BF16, kind="ExternalInput").ap()
    wpw_t = nc.dram_tensor("wpw_t", [C, C], BF16, kind="ExternalInput").ap()
    wpw_s = nc.dram_tensor("wpw_s", [C, C], BF16, kind="ExternalInput").ap()
    fw1_t = nc.dram_tensor("fw1", [2 * C, C], BF16, kind="ExternalInput").ap()
    fwf_t = nc.dram_tensor("fwf", [C, C], BF16, kind="ExternalInput").ap()
    idn_t = nc.dram_tensor("idn", [128, 128], BF16, kind="ExternalInput").ap()
    # vecs columns: 0-2 t_dw half0, 3-5 t_dw half1, 6-8 s_dw h0, 9-11 s_dw h1,
    # 12,13 t_bn_g h0/h1, 14,15 t_bn_b, 16,17 s_bn_g, 18,19 s_bn_b,
    # 22,23 final bias, 24 EPS
    vecs_t = nc.dram_tensor("vecs", [128, 26], F32, kind="ExternalInput").ap()
    out_t = nc.dram_tensor("out", [C, BSH * NTOK], F32, kind="ExternalOutput").ap()

    def dbg_out(name, shape, dtype=F32):
        ap = nc.dram_tensor("dbg_" + name, shape, dtype, kind="ExternalOutput").ap()
        dbg_outs.append("dbg_" + name)
        return ap

    tc = tile.TileContext(nc)
    with tc:
        _build_body(nc, tc, locals(), dbg, dbg_out)
    _split_excess_waits(nc)
    return dbg_outs


def _build_body(nc, tc, ctx, dbg, dbg_out):
    x_t = ctx["x_t"]; out_t = ctx["out_t"]; vecs_t = ctx["vecs_t"]
    wqkv_t = ctx["wqkv_t"]; wqkv_s = ctx["wqkv_s"]
    wproj_t = ctx["wproj_t"]; wproj_s = ctx["wproj_s"]
    wpw_t = ctx["wpw_t"]; wpw_s = ctx["wpw_s"]
    fw1_t = ctx["fw1_t"]; fwf_t = ctx["fwf_t"]
    idn_t = ctx["idn_t"]

    ex_cm = tc.tile_pool(name="ex", bufs=1)       # persistent: weights, stats, parks
    ex = ex_cm.__enter__()
    dr_cm = tc.tile_pool(name="dr", bufs=1, space="DRAM")
    dr = dr_cm.__enter__()

    # ---- persistent weight tiles ----
    def wload(name, src, n, w):
        ts = [ex.tile([128, w], BF16, name=f"{name}{k}") for k in range(n)]
        for k in range(n):
            nc.sync.dma_start(ts[k][:], src[128 * k:128 * (k + 1), :])
        return ts

    wqkvT = wload("wqkvT", wqkv_t, 2, 3 * C)
    wqkvS = wload("wqkvS", wqkv_s, 2, 3 * C)
    wprojT = wload("wprojT", wproj_t, 2, C)
    wprojS = wload("wprojS", wproj_s, 2, C)
    wpwT = wload("wpwT", wpw_t, 2, C)
    wpwS = wload("wpwS", wpw_s, 2, C)
    fw1T = wload("fw1T", fw1_t, 4, C)
    fwfT = wload("fwfT", fwf_t, 2, C)
    idn = ex.tile([128, 128], BF16, name="idn")
    nc.sync.dma_start(idn[:], idn_t[:])
    vecs = ex.tile([128, 26], F32, name="vecs")
    nc.sync.dma_start(vecs[:], vecs_t[:])

    # spatial pooled input, bf16, built during temporal loop
    xps = [ex.tile([128, BSH * TOKS_B], BF16, name=f"xps{k}") for k in range(2)]
    # BN partial accumulators: [t_sum h0,h1, t_sq h0,h1, s_sum h0,h1, s_sq h0,h1]
    accs = ex.tile([128, 8], F32, name="accs")
    nc.vector.memset(accs[:], 0.0)
    # temporal/spatial pre-BN activations parked in SBUF (bf16)
    yt_sb = [ex.tile([128, BSH * TOKT_B], BF16, name=f"yt_sb{k}") for k in range(2)]
    ys_sb = [ex.tile([128, BSH * TOKS_B], BF16, name=f"ys_sb{k}") for k in range(2)]

    # =================== PHASE A-t: temporal branch to pre-BN ===================
    with tc.tile_pool(name="pa", bufs=1) as pa, \
         tc.tile_pool(name="pa2", bufs=2) as pa2, \
         tc.tile_pool(name="pp", bufs=2, space="PSUM") as pp, \
         tc.tile_pool(name="pps", bufs=4, space="PSUM") as pps, \
         tc.tile_pool(name="ppo", bufs=2, space="PSUM") as ppo:
        for b in range(BSH):
            xc = [pa2.tile([128, NTOK], BF16, tag=f"xc{k}", name=f"xc{b}_{k}") for k in range(2)]
            xptb = [pa.tile([128, TOKT_B], BF16, tag=f"xptb{k}", name=f"xptb{b}_{k}") for k in range(2)]
            for k in range(2):
                nc.sync.dma_start(xc[k][:], x_t[128 * k:128 * (k + 1),
                                                b * NTOK:(b + 1) * NTOK])
                # temporal pool: out[c, j*81+m] = sum_r x[c, (3m+r)*17+j]
                xv = xc[k][:].rearrange("p (t j) -> p j t", j=J)
                xv = xv.rearrange("p j (m r) -> p j m r", r=3)
                with nc.allow_low_precision(reason="3-tap avg-pool in bf16"):
                    nc.vector.reduce_sum(xptb[k][:].rearrange("p (j m) -> p j m", j=J), xv, AX.X)
                # spatial pool: out[c, t*8 + l] = sum_r x[c, t*17 + 2l+r]
                xv2 = xc[k][:].rearrange("p (t j) -> p t j", t=T)[:, :, 0:16]
                xv2 = xv2.rearrange("p t (l r) -> p t l r", r=2)
                dst = xps[k][:, b * TOKS_B:(b + 1) * TOKS_B]
                with nc.allow_low_precision(reason="2-tap avg-pool in bf16"):
                    nc.vector.reduce_sum(dst.rearrange("p (t l) -> p t l", t=T), xv2, AX.X)

            # ---- temporal QKV (Q,K head-stacked [128, tok]; V token-major) ----
            qp = [pa.tile([128, TOKT_B], BF16, tag=f"qp{g}", name=f"qp{b}_{g}") for g in range(2)]
            kp = [pa.tile([128, TOKT_B], BF16, tag=f"kp{g}", name=f"kp{b}_{g}") for g in range(2)]
            chunks = [(0, 512), (512, 1024), (1024, TOKT_B)]
            for m in range(4):
                dstt = qp[m % 2] if m < 2 else kp[m % 2]
                for (c0, c1) in chunks:
                    ps = pp.tile([128, 512], F32, tag="pbig", name=f"qkps{b}_{m}_{c0}")
                    for k in range(2):
                        nc.tensor.matmul(
                            ps[:, :c1 - c0],
                            wqkvT[k][:, 128 * m:128 * (m + 1)],
                            xptb[k][:, c0:c1],
                            start=(k == 0), stop=(k == 1))
                    nc.scalar.copy(dstt[:, c0:c1], ps[:, :c1 - c0])
            vt = pa.tile([128, NT_B * 264], BF16, tag="vt", name=f"vt{b}")
            ones_ap = vt[:].rearrange("p (j h e) -> p j h e", j=NT_B, h=H)[:, :, :, 32]
            nc.vector.memset(ones_ap, 1.0)
            for j in range(NT_B):
                ps = pp.tile([128, 512], F32, tag="pbig", name=f"vps{b}_{j}")
                for k in range(2):
                    nc.tensor.matmul(
                        ps[:81, :256],
                        xptb[k][:, j * LT:(j + 1) * LT],
                        wqkvT[k][:, 512:768],
                        start=(k == 0), stop=(k == 1))
                dst = vt[:81, j * 264:(j + 1) * 264].rearrange("p (h e) -> p h e", h=H)[:, :, 0:32]
                nc.scalar.copy(dst, ps[:81, :256].rearrange("p (h d) -> p h d", h=H))

            # ---- attention per (j, h) ----
            otok = pa.tile([128, NT_B * C], BF16, tag="otok", name=f"otok{b}")
            for j in range(NT_B):
                pt = pa2.tile([128, 648], BF16, tag="pt", name=f"pt{b}_{j}")
                for h in range(H):
                    hg, hh = h // 4, h % 4
                    kk = kp[hg][32 * hh:32 * hh + 32, j * LT:(j + 1) * LT]
                    qq = qp[hg][32 * hh:32 * hh + 32, j * LT:(j + 1) * LT]
                    sp = pps.tile([128, 81], F32, tag="sp", name=f"sp{b}_{j}_{h}")
                    nc.tensor.matmul(sp[:81, :81], kk, qq, start=True, stop=True,
                                     tile_position=(32 * hh, 0))
                    nc.scalar.activation(pt[:81, 81 * h:81 * h + 81], sp[:81, :81],
                                         AF.Exp, scale=SCALE)
                rt = pa2.tile([128, 8], F32, tag="rt", name=f"rt{b}_{j}")
                for h in range(H):
                    opt = ppo.tile([128, 40], F32, tag="op", name=f"op{b}_{j}_{h}")
                    nc.tensor.matmul(
                        opt[:81, 0:33],
                        pt[:81, 81 * h:81 * h + 81],
                        vt[:81, j * 264 + 33 * h:j * 264 + 33 * h + 33],
                        start=True, stop=True)
                    nc.vector.reciprocal(rt[:81, h:h + 1], opt[:81, 32:33])
                    nc.scalar.activation(
                        otok[:81, j * C + 32 * h:j * C + 32 * h + 32],
                        opt[:81, 0:32],
                        AF.Copy, scale=rt[:81, h:h + 1])

            # ---- transpose O to channel-major ----
            ot = [pa.tile([128, TOKT_B], BF16, tag=f"ot{k}", name=f"ot{b}_{k}") for k in range(2)]
            for j in range(NT_B):
                for k in range(2):
                    pst = ppo.tile([128, 256], BF16, tag="op", name=f"tr{b}_{j}_{k}")
                    nc.tensor.transpose(pst[:128, :81], otok[:81, j * C + 128 * k:j * C + 128 * (k + 1)], idn[:81, :81])
                    nc.scalar.copy(ot[k][:, j * LT:(j + 1) * LT], pst[:128, :81])

            # ---- proj -> padded, dwconv, stats, park ----
            ypad = [pa.tile([128, NT_B * 83], F32, tag=f"ypad{m}", name=f"ypad{b}_{m}") for m in range(2)]
            for m in range(2):
                zv = ypad[m][:].rearrange("p (j s) -> p j s", j=NT_B)
                nc.vector.memset(zv[:, :, 0], 0.0)
                nc.vector.memset(zv[:, :, 82], 0.0)
            pchunks = [(0, 6), (6, 12), (12, 17)]
            for m in range(2):
                for (j0, j1) in pchunks:
                    ps = pp.tile([128, 512], F32, tag="pbig", name=f"pj{b}_{m}_{j0}")
                    w = (j1 - j0) * LT
                    for k in range(2):
                        nc.tensor.matmul(
                            ps[:, :w],
                            wprojT[k][:, 128 * m:128 * (m + 1)],
                            ot[k][:, j0 * LT:j1 * LT],
                            start=(k == 0), stop=(k == 1))
                    dst = ypad[m][:].rearrange("p (j s) -> p j s", j=NT_B)[:, j0:j1, 1:82]
                    nc.scalar.copy(dst, ps[:, :w].rearrange("p (j t) -> p j t", j=j1 - j0))
            ydw = [pa.tile([128, TOKT_B], F32, tag=f"ydw{m}", name=f"ydw{b}_{m}") for m in range(2)]
            scr = pa.tile([128, TOKT_B], F32, tag="scr", name=f"scr{b}")
            for m in range(2):
                zp = ypad[m][:].rearrange("p (j s) -> p j s", j=NT_B)
                yv = ydw[m][:].rearrange("p (j t) -> p j t", j=NT_B)
                dw = vecs[:, 3 * m:3 * m + 3]
                nc.vector.tensor_scalar_mul(yv, zp[:, :, 1:82], dw[:, 1:2])
                nc.vector.scalar_tensor_tensor(yv, zp[:, :, 0:81], dw[:, 0:1], yv, OP.mult, OP.add)
                nc.vector.scalar_tensor_tensor(yv, zp[:, :, 2:83], dw[:, 2:3], yv, OP.mult, OP.add)
                s1 = pa2.tile([128, 1], F32, tag="s1", name=f"s1{b}_{m}")
                nc.vector.reduce_sum(s1[:], ydw[m][:], AX.X)
                nc.vector.tensor_add(accs[:, m:m + 1], accs[:, m:m + 1], s1[:])
                s2 = pa2.tile([128, 1], F32, tag="s2", name=f"s2{b}_{m}")
                nc.scalar.activation(scr[:], ydw[m][:], AF.Square, accum_out=s2[:])
                nc.vector.tensor_add(accs[:, 2 + m:3 + m], accs[:, 2 + m:3 + m], s2[:])
                nc.scalar.copy(yt_sb[m][:, b * TOKT_B:(b + 1) * TOKT_B], ydw[m][:])
            if "ydw" in dbg and b == 0:
                d = dbg_out("ydw", [2, 128, TOKT_B])
                for m in range(2):
                    nc.sync.dma_start(d[m], ydw[m][:])

    # =================== PHASE A-s: spatial branch to pre-BN ===================
    if PHASES < 2:
        dr_cm.__exit__(None, None, None)
        ex_cm.__exit__(None, None, None)
        return
    nrows = [128, NS_B - 128]
    with tc.tile_pool(name="sa", bufs=1) as sa, \
         tc.tile_pool(name="sa2", bufs=2) as sa2, \
         tc.tile_pool(name="sq", bufs=4, space="PSUM") as sqp, \
         tc.tile_pool(name="st", bufs=2, space="PSUM") as stp, \
         tc.tile_pool(name="sp", bufs=2, space="PSUM") as spp:
        for b in range(BSH):
            # ---- seq-major QKV via per-l matmuls ----
            # qsm/ksm: [s, (h, l, d)]; vsm: [s, (h, d, l)]
            qsm = [sa.tile([128, H * 256], BF16, tag=f"qsm{t}", name=f"qsm{b}_{t}") for t in range(2)]
            ksm = [sa.tile([128, H * 256], BF16, tag=f"ksm{t}", name=f"ksm{b}_{t}") for t in range(2)]
            vsm = [sa.tile([128, H * 256], BF16, tag=f"vsm{t}", name=f"vsm{b}_{t}") for t in range(2)]
            for t in range(2):
                nr = nrows[t]
                t0 = 128 * t
                for l in range(LS):
                    for role in range(3):
                        ps = sqp.tile([128, 256], F32, tag="sq", name=f"sq{b}_{t}_{l}_{role}")
                        for k in range(2):
                            lhs = xps[k][:, b * TOKS_B:(b + 1) * TOKS_B].rearrange(
                                "p (t l) -> p t l", l=LS)[:, t0:t0 + nr, l]
                            nc.tensor.matmul(
                                ps[:nr, :],
                                lhs,
                                wqkvS[k][:, 256 * role:256 * (role + 1)],
                                start=(k == 0), stop=(k == 1))
                        if role < 2:
                            dstt = (qsm if role == 0 else ksm)[t]
                            dv = dstt[:nr].rearrange("s (h l d) -> s h l d", h=H, l=LS)[:, :, l, :]
                        else:
                            dv = vsm[t][:nr].rearrange("s (h d l) -> s h d l", h=H, l=LS)[:, :, :, l]
                        src = ps[:nr, :].rearrange("s (h d) -> s h d", h=H)
                        if role == 2:
                            nc.scalar.copy(dv, src)
                        else:
                            nc.vector.tensor_copy(dv, src)

            # ---- S = QK^T, softmax, O = PV (DVE broadcast ops, seq-major) ----
            osl = [sa.tile([128, 2048], BF16, tag=f"osl{t}", name=f"os{b}_{t}") for t in range(2)]
            for t in range(2):
                nr = nrows[t]
                sslab = sa2.tile([128, 512], F32, tag="sslab", name=f"ss{b}_{t}")
                prod = sa2.tile([128, 2048], BF16, tag="prod", name=f"pr{b}_{t}")
                for h in range(H):
                    q3 = qsm[t][:nr, 256 * h:256 * (h + 1)].rearrange("s (l d) -> s l d", l=LS)
                    k3 = ksm[t][:nr, 256 * h:256 * (h + 1)].rearrange("s (l d) -> s l d", l=LS)
                    qb = q3.unsqueeze(2).broadcast_to([nr, LS, LS, DH])
                    kb = k3.unsqueeze(1).broadcast_to([nr, LS, LS, DH])
                    pv = prod[:nr].rearrange("s (q k d) -> s q k d", q=LS, k=LS)
                    nc.vector.tensor_tensor(out=pv, in0=qb, in1=kb, op=OP.mult)
                    nc.vector.reduce_sum(
                        sslab[:nr, 64 * h:64 * (h + 1)].rearrange("s (q k) -> s q k", q=LS),
                        pv, AX.X)
                pslab = sa2.tile([128, 512], BF16, tag="pslab", name=f"pl{b}_{t}")
                nc.scalar.activation(pslab[:nr, :], sslab[:nr, :], AF.Exp, scale=SCALE)
                ssum = sa2.tile([128, 64], F32, tag="ssum", name=f"ssum{b}_{t}")
                nc.vector.reduce_sum(ssum[:nr, :],
                                     pslab[:nr].rearrange("s (hq k) -> s hq k", k=LS), AX.X)
                rcp = sa2.tile([128, 64], F32, tag="rcp", name=f"rcp{b}_{t}")
                nc.vector.reciprocal(rcp[:nr, :], ssum[:nr, :])
                rb = rcp[:nr].unsqueeze(2).broadcast_to([nr, 64, LS])
                p3v = pslab[:nr].rearrange("s (hq k) -> s hq k", k=LS)
                nc.vector.tensor_tensor(out=p3v, in0=p3v, in1=rb, op=OP.mult)
                for h in range(H):
                    p3 = pslab[:nr, 64 * h:64 * (h + 1)].rearrange("s (q k) -> s q k", q=LS)
                    pb = p3.unsqueeze(1).broadcast_to([nr, DH, LS, LS])
                    v3 = vsm[t][:nr, 256 * h:256 * (h + 1)].rearrange("s (d l) -> s d l", d=DH)
                    vb = v3.unsqueeze(2).broadcast_to([nr, DH, LS, LS])
                    pv2 = prod[:nr].rearrange("s (d q k) -> s d q k", d=DH, q=LS)
                    nc.vector.tensor_tensor(out=pv2, in0=pb, in1=vb, op=OP.mult)
                    with nc.allow_low_precision(reason="attn PV reduce to bf16"):
                        nc.vector.reduce_sum(
                            osl[t][:nr, 256 * h:256 * (h + 1)].rearrange("s (d q) -> s d q", d=DH),
                            pv2, AX.X)

            # ---- O back to channel-major via PE transposes (per q) ----
            ots = [sa.tile([128, TOKS_B], BF16, tag=f"ots{k}", name=f"ots{b}_{k}") for k in range(2)]
            for t in range(2):
                nr = nrows[t]
                for q in range(LS):
                    for kh in range(2):
                        src = osl[t][:nr].rearrange("s (h d q) -> s h d q", h=H, d=DH)[
                            :, 4 * kh:4 * kh + 4, :, q]
                        pst = stp.tile([128, 128], BF16, tag="st", name=f"st{b}_{t}_{q}_{kh}")
                        nc.tensor.transpose(pst[:128, :nr], src, idn[:nr, :nr])
                        dst = ots[kh][:].rearrange("p (t l) -> p t l", l=LS)[
                            :, 128 * t:128 * t + nr, q]
                        nc.scalar.copy(dst, pst[:128, :nr])
            if "ots" in dbg and b == 0:
                d = dbg_out("ots", [2, 128, TOKS_B], BF16)
                for k in range(2):
                    nc.sync.dma_start(d[k], ots[k][:])

            # ---- proj -> padded (10 per seq), dwconv over l, stats, park ----
            yspad = [sa.tile([128, NS_B * 10], F32, tag=f"yspad{m}", name=f"yspad{b}_{m}") for m in range(2)]
            for m in range(2):
                zv = yspad[m][:].rearrange("p (n s) -> p n s", n=NS_B)
                nc.vector.memset(zv[:, :, 0], 0.0)
                nc.vector.memset(zv[:, :, 9], 0.0)
            nchunks = [(0, 61), (61, 122), (122, 183), (183, 243)]
            for m in range(2):
                for (n0, n1) in nchunks:
                    ps = spp.tile([128, 512], F32, tag="spbig", name=f"sp{b}_{m}_{n0}")
                    w = (n1 - n0) * LS
                    for k in range(2):
                        nc.tensor.matmul(
                            ps[:, :w],
                            wprojS[k][:, 128 * m:128 * (m + 1)],
                            ots[k][:, n0 * LS:n1 * LS],
                            start=(k == 0), stop=(k == 1))
                    dst = yspad[m][:].rearrange("p (n s) -> p n s", n=NS_B)[:, n0:n1, 1:9]
                    nc.scalar.copy(dst, ps[:, :w].rearrange("p (n l) -> p n l", n=n1 - n0))
            for m in range(2):
                zp = yspad[m][:].rearrange("p (n s) -> p n s", n=NS_B)
                ydwt = sa.tile([128, TOKS_B], F32, tag=f"ysdw{m}", name=f"ysdw{b}_{m}")
                yv = ydwt[:].rearrange("p (n l) -> p n l", n=NS_B)
                dw = vecs[:, 6 + 3 * m:9 + 3 * m]
                nc.vector.tensor_scalar_mul(yv, zp[:, :, 1:9], dw[:, 1:2])
                nc.vector.scalar_tensor_tensor(yv, zp[:, :, 0:8], dw[:, 0:1], yv, OP.mult, OP.add)
                nc.vector.scalar_tensor_tensor(yv, zp[:, :, 2:10], dw[:, 2:3], yv, OP.mult, OP.add)
                s1 = sa2.tile([128, 1], F32, tag="ss1", name=f"ss1{b}_{m}")
                nc.vector.reduce_sum(s1[:], ydwt[:], AX.X)
                nc.vector.tensor_add(accs[:, 4 + m:5 + m], accs[:, 4 + m:5 + m], s1[:])
                scr2 = sa.tile([128, TOKS_B], F32, tag="sscr", name=f"sscr{b}_{m}")
                s2 = sa2.tile([128, 1], F32, tag="ss2", name=f"ss2{b}_{m}")
                nc.scalar.activation(scr2[:], ydwt[:], AF.Square, accum_out=s2[:])
                nc.vector.tensor_add(accs[:, 6 + m:7 + m], accs[:, 6 + m:7 + m], s2[:])
                nc.scalar.copy(ys_sb[m][:, b * TOKS_B:(b + 1) * TOKS_B], ydwt[:])

    # =================== PHASE B: AllReduce stats -> BN coefs ===================
    if PHASES < 3:
        dr_cm.__exit__(None, None, None)
        ex_cm.__exit__(None, None, None)
        return
    bnc = ex.tile([128, 8], F32, name="bnc")  # a_t h0,h1; b_t h0,h1; a_s h0,h1; b_s h0,h1
    with tc.tile_pool(name="pb", bufs=1) as pb:
        cin = dr.tile([128, 8], F32, name="cc_in")
        cout = dr.tile([128, 8], F32, name="cc_out")
        nc.sync.dma_start(cin[:], accs[:])
        nc.gpsimd.collective_compute(
            "AllReduce", OP.add,
            replica_groups=[list(range(N_CORES))],
            ins=[cin.opt()], outs=[cout.opt()])
        gst = pb.tile([128, 8], F32, name="gst")
        nc.sync.dma_start(gst[:], cout[:])
        tmp = pb.tile([128, 8], F32, name="btmp")
        for br, (cnt, sco, gco, bco) in enumerate(
                (((CNT_T), 0, 12, 14), ((CNT_S), 4, 16, 18))):
            for m in range(2):
                mu = pb.tile([128, 1], F32, tag="mu", name=f"mu{br}_{m}")
                nc.scalar.activation(mu[:], gst[:, sco + m:sco + m + 1], AF.Copy, scale=1.0 / cnt)
                m2 = pb.tile([128, 1], F32, tag="m2", name=f"m2{br}_{m}")
                nc.scalar.activation(m2[:], gst[:, sco + 2 + m:sco + 3 + m], AF.Copy, scale=1.0 / cnt)
                nc.vector.tensor_scalar(out=tmp[:, 0:1], in0=mu[:], scalar1=mu[:],
                                        scalar2=-1.0, op0=OP.mult, op1=OP.mult)
                nc.vector.tensor_add(tmp[:, 1:2], m2[:], tmp[:, 0:1])
                r = pb.tile([128, 1], F32, tag="rr", name=f"r{br}_{m}")
                nc.scalar.activation(tmp[:, 3:4], tmp[:, 1:2], AF.Sqrt, bias=vecs[:, 24:25])
                nc.vector.reciprocal(r[:], tmp[:, 3:4])
                a_col = 4 * br + m
                b_col = 4 * br + 2 + m
                nc.vector.tensor_tensor(out=bnc[:, a_col:a_col + 1],
                                        in0=vecs[:, gco + m:gco + m + 1], in1=r[:], op=OP.mult)
                nc.vector.tensor_tensor(out=tmp[:, 2:3], in0=mu[:],
                                        in1=bnc[:, a_col:a_col + 1], op=OP.mult)
                nc.vector.tensor_sub(bnc[:, b_col:b_col + 1],
                                     vecs[:, bco + m:bco + m + 1], tmp[:, 2:3])

    # =================== PHASE C: BN+GELU+pw+interp, fusion MLP ===================
    if PHASES < 4:
        dr_cm.__exit__(None, None, None)
        ex_cm.__exit__(None, None, None)
        return
    lo_s, hi_s, w_s = _interp_lin_coef(LS, J)
    with tc.tile_pool(name="ca", bufs=1) as caq, \
         tc.tile_pool(name="ca2", bufs=2) as ca2, \
         tc.tile_pool(name="cp", bufs=4, space="PSUM") as cp:
        for b in range(BSH):
            comb = [caq.tile([128, NTOK], BF16, tag=f"comb{q}", name=f"comb{b}_{q}") for q in range(4)]
            # ---------- temporal tail ----------
            gt = [caq.tile([128, TOKT_B], BF16, tag=f"gt{m}", name=f"gt{b}_{m}") for m in range(2)]
            for m in range(2):
                nc.scalar.activation(gt[m][:], yt_sb[m][:, b * TOKT_B:(b + 1) * TOKT_B],
                                     AF.Gelu, scale=bnc[:, m:m + 1], bias=bnc[:, 2 + m:3 + m])
            zpad = [caq.tile([128, NT_B * 83], F32, tag=f"zpad{m}", name=f"zpad{b}_{m}") for m in range(2)]
            pchunks = [(0, 6), (6, 12), (12, 17)]
            for m in range(2):
                for (j0, j1) in pchunks:
                    ps = cp.tile([128, 512], F32, tag="cbig", name=f"cpw{b}_{m}_{j0}")
                    w = (j1 - j0) * LT
                    for k in range(2):
                        nc.tensor.matmul(
                            ps[:, :w],
                            wpwT[k][:, 128 * m:128 * (m + 1)],
                            gt[k][:, j0 * LT:j1 * LT],
                            start=(k == 0), stop=(k == 1))
                    dst = zpad[m][:].rearrange("p (j s) -> p j s", j=NT_B)[:, j0:j1, 1:82]
                    nc.scalar.copy(dst, ps[:, :w].rearrange("p (j t) -> p j t", j=j1 - j0))
                zv = zpad[m][:].rearrange("p (j s) -> p j s", j=NT_B)
                nc.vector.tensor_copy(zv[:, :, 0], zv[:, :, 1])
                nc.vector.tensor_copy(zv[:, :, 82], zv[:, :, 81])
                z23 = ca2.tile([128, TOKT_B], F32, tag="z23", name=f"z23{b}_{m}")
                nc.scalar.activation(z23[:].rearrange("p (j t) -> p j t", j=NT_B),
                                     zv[:, :, 1:82], AF.Copy, scale=2.0 / 3.0)
                z23v = z23[:].rearrange("p (j t) -> p j t", j=NT_B)
                dst1 = _interp_dst(comb[m], 1)
                nc.vector.tensor_copy(dst1, zv[:, :, 1:82])
                dst0 = _interp_dst(comb[m], 0)
                nc.vector.scalar_tensor_tensor(dst0, zv[:, :, 0:81], 1.0 / 3.0, z23v, OP.mult, OP.add)
                dst2 = _interp_dst(comb[m], 2)
                nc.vector.scalar_tensor_tensor(dst2, zv[:, :, 2:83], 1.0 / 3.0, z23v, OP.mult, OP.add)
            # ---------- spatial tail ----------
            gs = [caq.tile([128, TOKS_B], BF16, tag=f"gs{m}", name=f"gs{b}_{m}") for m in range(2)]
            for m in range(2):
                nc.scalar.activation(gs[m][:], ys_sb[m][:, b * TOKS_B:(b + 1) * TOKS_B],
                                     AF.Gelu, scale=bnc[:, 4 + m:5 + m], bias=bnc[:, 6 + m:7 + m])
            zs = [caq.tile([128, TOKS_B], F32, tag=f"zs{m}", name=f"zs{b}_{m}") for m in range(2)]
            nchunks = [(0, 61), (61, 122), (122, 183), (183, 243)]
            for m in range(2):
                for (n0, n1) in nchunks:
                    ps = cp.tile([128, 512], F32, tag="cbig", name=f"cps{b}_{m}_{n0}")
                    w = (n1 - n0) * LS
                    for k in range(2):
                        nc.tensor.matmul(
                            ps[:, :w],
                            wpwS[k][:, 128 * m:128 * (m + 1)],
                            gs[k][:, n0 * LS:n1 * LS],
                            start=(k == 0), stop=(k == 1))
                    nc.scalar.copy(zs[m][:, n0 * LS:n1 * LS], ps[:, :w])
                zsv = zs[m][:].rearrange("p (n l) -> p n l", n=NS_B)
                cmv = comb[2 + m][:].rearrange("p (t j) -> p t j", t=T)
                for jj in range(J):
                    lo, hi, w = int(lo_s[jj]), int(hi_s[jj]), float(w_s[jj])
                    if w < 1e-9 or lo == hi:
                        nc.scalar.copy(cmv[:, :, jj], zsv[:, :, lo])
                    else:
                        nc.scalar.activation(cmv[:, :, jj], zsv[:, :, lo], AF.Copy, scale=1.0 - w)
                        nc.vector.scalar_tensor_tensor(cmv[:, :, jj], zsv[:, :, hi], w,
                                                       cmv[:, :, jj], OP.mult, OP.add)
            if "comb" in dbg and b == 0:
                d = dbg_out("comb", [4, 128, NTOK], BF16)
                for q in range(4):
                    nc.sync.dma_start(d[q], comb[q][:])

            # ---------- fusion MLP ----------
            g1T = [caq.tile([128, 4144], BF16, tag=f"g1T{k}", name=f"g1T{b}_{k}") for k in range(2)]
            tchunks = [(i * 128, min(NTOK, (i + 1) * 128)) for i in range((NTOK + 127) // 128)]
            for (t0, t1) in tchunks:
                tl = t1 - t0
                ps = cp.tile([128, 256], F32, tag="ch1", name=f"h1{b}_{t0}")
                for q in range(4):
                    nc.tensor.matmul(ps[:tl, :], comb[q][:, t0:t1], fw1T[q][:],
                                     start=(q == 0), stop=(q == 3))
                bnst = ca2.tile([128, 6], F32, tag="bnst", name=f"bnst{b}_{t0}")
                nc.vector.bn_stats(bnst[:tl, :], ps[:tl, :])
                bnag = ca2.tile([128, 2], F32, tag="bnag", name=f"bnag{b}_{t0}")
                nc.vector.bn_aggr(bnag[:tl, :], bnst[:tl, :])
                r = ca2.tile([128, 1], F32, tag="lr", name=f"lr{b}_{t0}")
                sdv = ca2.tile([128, 1], F32, tag="sdv", name=f"sdv{b}_{t0}")
                nc.scalar.activation(sdv[:tl, :], bnag[:tl, 1:2], AF.Sqrt, bias=vecs[:tl, 24:25])
                nc.vector.reciprocal(r[:tl, :], sdv[:tl, :])
                nmr = ca2.tile([128, 1], F32, tag="nmr", name=f"nmr{b}_{t0}")
                nc.vector.tensor_scalar(out=nmr[:tl, :], in0=bnag[:tl, 0:1],
                                        scalar1=r[:tl, :], scalar2=-1.0,
                                        op0=OP.mult, op1=OP.mult)
                g1c = ca2.tile([128, 256], BF16, tag="g1c", name=f"g1c{b}_{t0}")
                nc.scalar.activation(g1c[:tl, :], ps[:tl, :], AF.Gelu,
                                     scale=r[:tl, :], bias=nmr[:tl, :])
                tpad = (tl + 15) // 16 * 16
                for k in range(2):
                    nc.sync.dma_start_transpose(g1T[k][:, t0:t0 + tpad],
                                                g1c[:tpad, 128 * k:128 * (k + 1)])
            # final (fused f_w2 @ p_w) -> channel-major f32 out
            fchunks = [(i * 512, min(NTOK, (i + 1) * 512)) for i in range((NTOK + 511) // 512)]
            for m in range(2):
                for (c0, c1) in fchunks:
                    ps = cp.tile([128, 512], F32, tag="cbig", name=f"hf{b}_{m}_{c0}")
                    for k in range(2):
                        nc.tensor.matmul(ps[:, :c1 - c0], fwfT[k][:, 128 * m:128 * (m + 1)],
                                         g1T[k][:, c0:c1], start=(k == 0), stop=(k == 1))
                    ol = ca2.tile([128, 512], F32, tag="ol", name=f"ol{b}_{m}_{c0}")
                    nc.scalar.activation(ol[:, :c1 - c0], ps[:, :c1 - c0], AF.Identity,
                                         bias=vecs[:, 22 + m:23 + m])
                    nc.sync.dma_start(
                        out_t[128 * m:128 * (m + 1), b * NTOK + c0:b * NTOK + c1],
                        ol[:, :c1 - c0])

    dr_cm.__exit__(None, None, None)
    ex_cm.__exit__(None, None, None)


def _interp_dst(comb, delta):
    # comb [128, (t j)]: col(t=3mm+delta, j) = mm*51 + delta*17 + j
    v = comb[:].rearrange("p (mm s) -> p mm s", s=51)[:, :, delta * J:(delta + 1) * J]
    return v.transpose([0, 2, 1])  # -> [128, j:17, mm:81] to match z views


def prep_inputs(x, t_qkv, t_proj, t_dw, t_bn_g, t_bn_b, t_pw,
                s_qkv, s_proj, s_dw, s_bn_g, s_bn_b, s_pw,
                f_w1, f_b1, f_ln_g, f_ln_b, f_w2, f_b2, p_w, p_b):
    """Host-side: build per-core in_maps."""
    bf = ml_dtypes.bfloat16
    assert np.all(np.asarray(f_ln_g) == 1.0) and np.all(np.asarray(f_ln_b) == 0.0), \
        "general LayerNorm affine not emitted in this build"
    assert np.all(np.asarray(f_b1) == 0.0), "f_b1 != 0 not emitted in this build"
    wf = np.asarray(p_w) @ np.asarray(f_w2)           # fused final weight [C,C]
    bf2 = np.asarray(p_w) @ np.asarray(f_b2) + np.asarray(p_b)
    common = {
        "wqkv_t": np.ascontiguousarray((np.asarray(t_qkv).T / 3.0).astype(bf)),
        "wqkv_s": np.ascontiguousarray((np.asarray(s_qkv).T / 2.0).astype(bf)),
        "wproj_t": np.ascontiguousarray(np.asarray(t_proj).T.astype(bf)),
        "wproj_s": np.ascontiguousarray(np.asarray(s_proj).T.astype(bf)),
        "wpw_t": np.ascontiguousarray(np.asarray(t_pw).T.astype(bf)),
        "wpw_s": np.ascontiguousarray(np.asarray(s_pw).T.astype(bf)),
        "fw1": np.ascontiguousarray(np.asarray(f_w1).T.astype(bf)),
        "fwf": np.ascontiguousarray(wf.T.astype(bf)),
        "idn": np.eye(128, dtype=bf),
    }
    vecs = np.zeros((128, 26), np.float32)
    vecs[:, 24] = EPS
    tdw = np.asarray(t_dw).reshape(C, 3)
    sdw = np.asarray(s_dw).reshape(C, 3)
    for m in range(2):
        vecs[:, 3 * m:3 * m + 3] = tdw[128 * m:128 * (m + 1)]
        vecs[:, 6 + 3 * m:9 + 3 * m] = sdw[128 * m:128 * (m + 1)]
        vecs[:, 12 + m] = np.asarray(t_bn_g)[128 * m:128 * (m + 1)]
        vecs[:, 14 + m] = np.asarray(t_bn_b)[128 * m:128 * (m + 1)]
        vecs[:, 16 + m] = np.asarray(s_bn_g)[128 * m:128 * (m + 1)]
        vecs[:, 18 + m] = np.asarray(s_bn_b)[128 * m:128 * (m + 1)]
        vecs[:, 22 + m] = bf2[128 * m:128 * (m + 1)]
    common["vecs"] = vecs
    x = np.asarray(x, dtype=np.float32)
    in_maps = []
    for i in range(N_CORES):
        m = dict(common)
        xsl = x[i * BSH:(i + 1) * BSH]                     # [4, T, J, C]
        m["xs"] = np.ascontiguousarray(
            xsl.transpose(3, 0, 1, 2).reshape(C, BSH * NTOK).astype(bf))
        in_maps.append(m)
    return in_maps


_BUILD_CACHE = {}


def _get_nc(dbg=()):
    key = tuple(sorted(dbg))
    if key not in _BUILD_CACHE:
        nc = bass.Bass(trn_type="TRN2", target_bir_lowering=False, num_devices=N_CORES)
        dbg_outs = build(nc, dbg)
        _BUILD_CACHE[key] = (nc, dbg_outs)
    return _BUILD_CACHE[key]


def run(inputs, dbg=(), trace=False, tmpdir=None):
    nc, dbg_outs = _get_nc(dbg)
    in_maps = prep_inputs(**inputs)
    res = run_bass_kernel_spmd(nc, in_maps, core_ids=list(range(N_CORES)), trace=trace,
                               tmpdir=tmpdir)
    outs = []
    for r in res.results:
        o = r["out"].reshape(C, BSH, T, J).transpose(1, 2, 3, 0)  # [4,T,J,C]
        outs.append(o)
    full = np.concatenate(outs, axis=0)
    return np.ascontiguousarray(full), res, dbg_outs


def kernel(**inputs) -> np.ndarray:
    full, _, _ = run(inputs)
    return full.astype(np.float32)
